# revision 10
# baseline (speedup 1.0000x reference)
"""Trainium2 Bass kernel for nn_CoordsToNRF.

out[b, p] = atom_nc[b, p] * (AU2KCALMOLA / MAX_NRF) / ||coords[b, I[p]] - coords[b, J[p]]||^2

Design (pure data parallel over batch, 8 cores x 128 batches):
  - Pair gather+subtract on the TensorEngine per xyz component:
        D_c = CT_c.T @ S        (S = +1/-1 tril pair-selection matrix)
    Coords use a TWO-term f16 split (C = C0 + C1/2^14, ~22-bit effective)
    accumulated in f32 PSUM: 6 matmuls per 512-pair group (96 total) vs
    the 3-term baseline's 144.  S is stored in HBM as fp8 (e4m3 for the
    +-1 plane, e5m2 for the exact +-2^-14 scaled plane) and fed to the
    PE directly as the fp8 moving operand: halves smat HBM traffic.
  - PE prewarm: ~10 dummy matmuls issued before any DMA lands so the HAM
    clock gate reaches 2.4 GHz before real work (baseline ran 19us cold).
  - ScalarE: one activation per group squares all 3 planes ([128,3,512]
    PSUM read across 3 banks) with the output scale folding 2/K; bf16 out.
  - VectorE: two adds (bf16 in, f32 out) + ONE fused scalar_tensor_tensor
      out = (anc * 2) / r2   (op0=mult, op1=divide)  -> bf16
    replacing the recip+mul pair.
  - IO dtypes: atom_nc uploaded f16, output downloaded bf16 and upcast on
    host (f16 would overflow: max out ~3.5e6).  Simulated end-to-end max
    rel err on the seed-0 data: ~1.1e-2 vs the 2e-2 harness gate.
"""

import sys

for _p in ("/opt/trn_rl_repo",):
    if _p not in sys.path:
        sys.path.insert(0, _p)

import numpy as np
import ml_dtypes
from contextlib import ExitStack

import concourse.bass as bass
import concourse.bacc as bacc
import concourse.tile as tile
from concourse import mybir
from concourse.bass_utils import run_bass_kernel_spmd

F32 = mybir.dt.float32
F16 = mybir.dt.float16
BF16 = mybir.dt.bfloat16
FP8E4 = mybir.dt.float8e4
FP8E5 = mybir.dt.float8e5

N_ATOMS = 128
NC2 = N_ATOMS * (N_ATOMS - 1) // 2  # 8128
BATCH = 1024
N_CORES = 8
BPC = BATCH // N_CORES  # 128 batches per core

AU2KCALMOLA = 627.5095 * 0.529177
MAX_NRF = 13036.0
K_CONST = AU2KCALMOLA / MAX_NRF
# fold 2/K into the squares so the final step is (anc*2)/r2'
SQ_SCALE = float(np.sqrt(2.0 / K_CONST))
LO_SHIFT = 2.0 ** 14

GROUP = 512
GROUPS = [(g, min(GROUP, NC2 - g)) for g in range(0, NC2, GROUP)]
CHUNK = 2048  # input-load chunk for smat/anc
CHUNKS = [(c, min(CHUNK, NC2 - c)) for c in range(0, NC2, CHUNK)]
OUT_TILE = 1024  # output store granularity (2 groups)

_I, _J = np.tril_indices(N_ATOMS, -1)


def _build_smat8():
    """S as fp8: hi plane e4m3 (+-1), lo plane e5m2 (+-2^-14), both exact."""
    hi = np.zeros((N_ATOMS, NC2), dtype=np.float32)
    p = np.arange(NC2)
    hi[_I, p] = 1.0
    hi[_J, p] = -1.0
    lo = hi * (1.0 / LO_SHIFT)
    return hi.astype(ml_dtypes.float8_e4m3fn), lo.astype(ml_dtypes.float8_e5m2)


def _split_coords2(coords32: np.ndarray):
    """[B, A, 3] f32 -> two f16 terms with C ~= C0 + C1/2^14 (~22-bit)."""
    c64 = coords32.astype(np.float64)
    c0 = c64.astype(np.float16)
    r1 = (c64 - c0.astype(np.float64)) * LO_SHIFT
    c1 = r1.astype(np.float16)
    return c0, c1


# feature flags (fallbacks for HW/compiler limitations)
USE_FP8_MOVING = True    # feed fp8 smat directly as the matmul moving operand
USE_STT_DIVIDE = False   # divide fails the walrus ISA check -> recip+mul
USE_ACT3 = True          # one 3-plane [128,3,512] activation per group
USE_MIXED_ADD = True     # bf16-in/f32-out tensor_add
USE_PREWARM = True


def _build_program():
    nc = bacc.Bacc("TRN2", target_bir_lowering=False, debug=False)

    # pre-transposed coords stationaries: [atoms, 3*batch] per term
    s_dt = FP8E4 if USE_FP8_MOVING else F16
    s_dt_lo = FP8E5 if USE_FP8_MOVING else F16
    ct_d = [
        nc.dram_tensor(f"ct{t}", [N_ATOMS, 3 * BPC], F16, kind="ExternalInput")
        for t in range(2)
    ]
    shi_d = nc.dram_tensor("smat_hi", [N_ATOMS, NC2], s_dt, kind="ExternalInput")
    slo_d = nc.dram_tensor("smat_lo", [N_ATOMS, NC2], s_dt_lo, kind="ExternalInput")
    anc_d = nc.dram_tensor("atom_nc", [BPC, NC2], F16, kind="ExternalInput")
    out_d = nc.dram_tensor("out", [BPC, NC2], BF16, kind="ExternalOutput")

    with tile.TileContext(nc) as tc, ExitStack() as ctx:
        const = ctx.enter_context(tc.tile_pool(name="const", bufs=1))
        work = ctx.enter_context(tc.tile_pool(name="work", bufs=4))
        outp = ctx.enter_context(tc.tile_pool(name="outp", bufs=3))
        ps_w = ctx.enter_context(tc.tile_pool(name="ps_w", bufs=1, space="PSUM"))
        ps_d = ctx.enter_context(tc.tile_pool(name="ps_d", bufs=2, space="PSUM"))

        # ---- PE prewarm: dummy matmuls so HAM un-throttles (~3.4us busy)
        # before the first real matmul; runs while input DMAs are in flight.
        if USE_PREWARM:
            junk = const.tile([128, GROUP], F16, tag="junk")
            nc.vector.memset(junk[:], 0)
            pw = ps_w.tile([128, GROUP], F32)
            for _ in range(10):
                nc.tensor.matmul(
                    pw[:], junk[:, :128], junk[:], start=True, stop=True,
                    skip_group_check=True,
                )

        # ---- inputs ----
        ct_sb = []
        for t in range(2):
            cs = const.tile([N_ATOMS, 3, BPC], F16, tag=f"ct{t}")
            nc.sync.dma_start(cs[:], ct_d[t][:, :].rearrange("a (c b) -> a c b", c=3))
            ct_sb.append(cs)

        shi_sb, slo_sb, anc_sb = [], [], []
        for ci, (c0, cw) in enumerate(CHUNKS):
            hi = const.tile([N_ATOMS, cw], s_dt, tag=f"shi{ci}")
            nc.sync.dma_start(hi[:], shi_d[:, c0:c0 + cw])
            shi_sb.append(hi)
            lo = const.tile([N_ATOMS, cw], s_dt_lo, tag=f"slo{ci}")
            nc.sync.dma_start(lo[:], slo_d[:, c0:c0 + cw])
            slo_sb.append(lo)
            at = const.tile([BPC, cw], F16, tag=f"anc{ci}")
            nc.sync.dma_start(at[:], anc_d[:, c0:c0 + cw])
            anc_sb.append(at)

        # ---- main loop: one 512-pair group at a time ----
        o_tile = None
        for gi, (gs, fd) in enumerate(GROUPS):
            ci, off = gs // CHUNK, gs % CHUNK

            d3 = ps_d.tile([128, 3, GROUP], F32, tag="d3")
            for c in range(3):
                for t in range(2):
                    rhs_pool = shi_sb if t == 0 else slo_sb
                    nc.tensor.matmul(
                        d3[:, c, :fd], ct_sb[t][:, c, :],
                        rhs_pool[ci][:, off:off + fd],
                        start=(t == 0), stop=(t == 1),
                        skip_group_check=True,
                    )

            # all 3 squares in one ACT pass, 2/K folded into the scale
            sq_dt = BF16 if USE_MIXED_ADD else F32
            sq = work.tile([128, 3, GROUP], sq_dt, tag="sq")
            if USE_ACT3:
                nc.scalar.activation(
                    sq[:, :, :fd], d3[:, :, :fd],
                    mybir.ActivationFunctionType.Square,
                    bias=0.0, scale=SQ_SCALE,
                )
            else:
                for c in range(3):
                    nc.scalar.activation(
                        sq[:, c, :fd], d3[:, c, :fd],
                        mybir.ActivationFunctionType.Square,
                        bias=0.0, scale=SQ_SCALE,
                    )

            t01 = work.tile([128, GROUP], F32, tag="t01")
            nc.vector.tensor_add(t01[:, :fd], sq[:, 0, :fd], sq[:, 1, :fd])
            r2 = work.tile([128, GROUP], F32, tag="r2")
            nc.vector.tensor_add(r2[:, :fd], t01[:, :fd], sq[:, 2, :fd])

            # fused (anc*2)/r2 -> bf16, into the 1024-wide output tile
            if gi % 2 == 0:
                o_tile = outp.tile([128, OUT_TILE], BF16)
            po = (gs % OUT_TILE)
            if USE_STT_DIVIDE:
                nc.vector.scalar_tensor_tensor(
                    o_tile[:, po:po + fd],
                    anc_sb[ci][:, off:off + fd], 2.0, r2[:, :fd],
                    mybir.AluOpType.mult, mybir.AluOpType.divide,
                )
            else:
                inv = work.tile([128, GROUP], F32, tag="inv")
                nc.vector.reciprocal_approx_fast(inv[:, :fd], r2[:, :fd])
                nc.vector.scalar_tensor_tensor(
                    o_tile[:, po:po + fd],
                    anc_sb[ci][:, off:off + fd], 2.0, inv[:, :fd],
                    mybir.AluOpType.mult, mybir.AluOpType.mult,
                )
            if gi % 2 == 1 or gs + fd == NC2:
                ms = (gs // OUT_TILE) * OUT_TILE
                mw = min(OUT_TILE, NC2 - ms)
                nc.sync.dma_start(out_d[:, ms:ms + mw], o_tile[:, :mw])

    nc.compile()
    return nc


_CACHED = None


def _get_program():
    global _CACHED
    if _CACHED is None:
        _CACHED = _build_program()
    return _CACHED


def kernel(coords, atom_nc, _trace=False, _trace_kwargs=None):
    coords = np.ascontiguousarray(np.asarray(coords, dtype=np.float32))
    atom_nc = np.ascontiguousarray(np.asarray(atom_nc, dtype=np.float32))
    assert coords.shape == (BATCH, N_ATOMS, 3)
    assert atom_nc.shape == (BATCH, NC2)

    nc = _get_program()
    if USE_FP8_MOVING:
        smat_hi, smat_lo = _build_smat8()
    else:
        hi = np.zeros((N_ATOMS, NC2), dtype=np.float32)
        p = np.arange(NC2)
        hi[_I, p] = 1.0
        hi[_J, p] = -1.0
        smat_hi = hi.astype(np.float16)
        smat_lo = (hi * (1.0 / LO_SHIFT)).astype(np.float16)
    c0, c1 = _split_coords2(coords)
    anc16 = atom_nc.astype(np.float16)

    in_maps = []
    for core in range(N_CORES):
        b0 = core * BPC
        # [batch, atoms, 3] f16 -> [atoms, 3*batch] (pre-transposed stationary)
        ct0 = np.ascontiguousarray(
            c0[b0:b0 + BPC].transpose(1, 2, 0).reshape(N_ATOMS, 3 * BPC))
        ct1 = np.ascontiguousarray(
            c1[b0:b0 + BPC].transpose(1, 2, 0).reshape(N_ATOMS, 3 * BPC))
        in_maps.append({
            "ct0": ct0,
            "ct1": ct1,
            "smat_hi": smat_hi,
            "smat_lo": smat_lo,
            "atom_nc": anc16[b0:b0 + BPC],
        })

    kw = {}
    if _trace:
        kw["trace"] = True
        kw.update(_trace_kwargs or {})
    res = run_bass_kernel_spmd(nc, in_maps, core_ids=list(range(N_CORES)), **kw)
    out = np.concatenate(
        [np.asarray(r["out"]).astype(np.float32) for r in res.results], axis=0)
    if _trace:
        return out, res
    return out


if __name__ == "__main__":
    rng = np.random.default_rng(0)
    coords = (rng.standard_normal((BATCH, N_ATOMS, 3)) * 5.0).astype(np.float32)
    atom_nc = rng.uniform(1.0, 50.0, (BATCH, NC2)).astype(np.float32)
    out = kernel(coords, atom_nc)
    print(out.shape, out.dtype)


# revision 11
# speedup vs baseline: 1.1940x; 1.1940x over previous
"""Trainium2 Bass kernel for nn_CoordsToNRF.

out[b, p] = atom_nc[b, p] * (AU2KCALMOLA / MAX_NRF) / ||coords[b, I[p]] - coords[b, J[p]]||^2

Design (pure data parallel over batch, 8 cores x 128 batches):
  - Pair gather+subtract on the TensorEngine per xyz component:
        D_c = CT_c.T @ S        (S = +-2^-7 tril pair-selection matrix)
    Coords use a TWO-term f16 split accumulated in f32 PSUM: the shared
    smat plane holds +-2^-7 and the per-term scales are folded into the
    f16 coords stationaries (ct0*2^7 exact, ct1*2^-7).  6 matmuls per
    512-pair group (96 total) vs the 3-term baseline's 144.
  - PE prewarm: dummy matmuls issued before any DMA lands so the HAM
    clock gate reaches 2.4 GHz before real work (baseline ran 19us cold).
  - ScalarE: one activation per group squares all 3 planes ([128,3,512]
    PSUM read across 3 banks) with the output scale folding 2/K; bf16 out.
  - VectorE: add1 bf16 (2x mode), add2 -> f32, reciprocal_approx_fast.
  - GpSimd: final anc*inv multiply (anc pre-doubled on host), bf16 out.
  - IO dtypes: atom_nc uploaded f16, output downloaded bf16 and upcast on
    host (f16 would overflow: max out ~3.5e6).  Simulated end-to-end max
    rel err on the seed-0 data: ~1.1e-2 vs the 2e-2 harness gate.
"""

import sys

for _p in ("/opt/trn_rl_repo",):
    if _p not in sys.path:
        sys.path.insert(0, _p)

import numpy as np
import ml_dtypes
from contextlib import ExitStack

import concourse.bass as bass
import concourse.bacc as bacc
import concourse.tile as tile
from concourse import mybir
from concourse.bass_utils import run_bass_kernel_spmd

F32 = mybir.dt.float32
F16 = mybir.dt.float16
BF16 = mybir.dt.bfloat16

N_ATOMS = 128
NC2 = N_ATOMS * (N_ATOMS - 1) // 2  # 8128
BATCH = 1024
N_CORES = 8
BPC = BATCH // N_CORES  # 128 batches per core

AU2KCALMOLA = 627.5095 * 0.529177
MAX_NRF = 13036.0
K_CONST = AU2KCALMOLA / MAX_NRF
# fold 2/K into the squares so the final step is (2*anc)*(1/r2')
SQ_SCALE = float(np.sqrt(2.0 / K_CONST))
LO_SHIFT = 2.0 ** 14
S_SCALE = 2.0 ** -7  # value stored in the shared smat plane

GROUP = 512
GROUPS = [(g, min(GROUP, NC2 - g)) for g in range(0, NC2, GROUP)]
CHUNK = 2048  # input-load chunk for smat/anc
CHUNKS = [(c, min(CHUNK, NC2 - c)) for c in range(0, NC2, CHUNK)]
OUT_TILE = 1024  # output store granularity (2 groups)

_I, _J = np.tril_indices(N_ATOMS, -1)

# feature flags
USE_PREWARM = True
MUL_ON_GPSIMD = True


def _build_smat16() -> np.ndarray:
    s = np.zeros((N_ATOMS, NC2), dtype=np.float32)
    p = np.arange(NC2)
    s[_I, p] = S_SCALE
    s[_J, p] = -S_SCALE
    return s.astype(np.float16)


def _split_coords2(coords32: np.ndarray):
    """[B, A, 3] f32 -> two f16 stationary terms with the smat's 2^-7
    compensated: (ct0*2^7 + ct1*2^-7) * 2^-7 ~= coords (~22-bit)."""
    c64 = coords32.astype(np.float64)
    c0 = c64.astype(np.float16)
    r1 = (c64 - c0.astype(np.float64)) * LO_SHIFT
    c1 = r1.astype(np.float16)
    ct0p = (c0.astype(np.float32) * 2.0 ** 7).astype(np.float16)
    ct1p = (c1.astype(np.float32) * 2.0 ** -7).astype(np.float16)
    return ct0p, ct1p


def _build_program():
    nc = bacc.Bacc("TRN2", target_bir_lowering=False, debug=False)

    ct_d = [
        nc.dram_tensor(f"ct{t}", [N_ATOMS, 3 * BPC], F16, kind="ExternalInput")
        for t in range(2)
    ]
    smat_d = nc.dram_tensor("smat", [N_ATOMS, NC2], F16, kind="ExternalInput")
    anc_d = nc.dram_tensor("atom_nc", [BPC, NC2], F16, kind="ExternalInput")
    out_d = nc.dram_tensor("out", [BPC, NC2], BF16, kind="ExternalOutput")

    with tile.TileContext(nc) as tc, ExitStack() as ctx:
        const = ctx.enter_context(tc.tile_pool(name="const", bufs=1))
        work = ctx.enter_context(tc.tile_pool(name="work", bufs=4))
        outp = ctx.enter_context(tc.tile_pool(name="outp", bufs=3))
        ps_w = ctx.enter_context(tc.tile_pool(name="ps_w", bufs=1, space="PSUM"))
        ps_d = ctx.enter_context(tc.tile_pool(name="ps_d", bufs=2, space="PSUM"))

        # ---- PE prewarm: dummy matmuls so HAM un-throttles (~3.4us busy)
        # before the first real matmul; runs while input DMAs are in flight.
        if USE_PREWARM:
            junk = const.tile([128, GROUP], F16, tag="junk")
            nc.vector.memset(junk[:], 0)
            pw = ps_w.tile([128, GROUP], F32)
            for _ in range(10):
                nc.tensor.matmul(
                    pw[:], junk[:, :128], junk[:], start=True, stop=True,
                    skip_group_check=True,
                )

        # ---- inputs ----
        ct_sb = []
        for t in range(2):
            cs = const.tile([N_ATOMS, 3, BPC], F16, tag=f"ct{t}")
            nc.sync.dma_start(cs[:], ct_d[t][:, :].rearrange("a (c b) -> a c b", c=3))
            ct_sb.append(cs)

        smat_sb, anc_sb = [], []
        for ci, (c0, cw) in enumerate(CHUNKS):
            st = const.tile([N_ATOMS, cw], F16, tag=f"smat{ci}")
            nc.sync.dma_start(st[:], smat_d[:, c0:c0 + cw])
            smat_sb.append(st)
            at = const.tile([BPC, cw], F16, tag=f"anc{ci}")
            nc.sync.dma_start(at[:], anc_d[:, c0:c0 + cw])
            anc_sb.append(at)

        # ---- main loop: one 512-pair group at a time ----
        o_tile = None
        for gi, (gs, fd) in enumerate(GROUPS):
            ci, off = gs // CHUNK, gs % CHUNK

            d3 = ps_d.tile([128, 3, GROUP], F32, tag="d3")
            for c in range(3):
                for t in range(2):
                    nc.tensor.matmul(
                        d3[:, c, :fd], ct_sb[t][:, c, :],
                        smat_sb[ci][:, off:off + fd],
                        start=(t == 0), stop=(t == 1),
                        skip_group_check=True,
                    )

            # all 3 squares in one ACT pass, 2/K folded into the scale
            sq = work.tile([128, 3, GROUP], BF16, tag="sq")
            nc.scalar.activation(
                sq[:, :, :fd], d3[:, :, :fd],
                mybir.ActivationFunctionType.Square,
                bias=0.0, scale=SQ_SCALE,
            )

            t01 = work.tile([128, GROUP], BF16, tag="t01")
            nc.vector.tensor_add(t01[:, :fd], sq[:, 0, :fd], sq[:, 1, :fd])
            r2 = work.tile([128, GROUP], F32, tag="r2")
            nc.vector.tensor_add(r2[:, :fd], t01[:, :fd], sq[:, 2, :fd])
            inv = work.tile([128, GROUP], F32, tag="inv")
            nc.vector.reciprocal_approx_fast(inv[:, :fd], r2[:, :fd])

            # final multiply on gpsimd (anc pre-doubled host-side)
            if gi % 2 == 0:
                o_tile = outp.tile([128, OUT_TILE], BF16)
            po = (gs % OUT_TILE)
            mul_eng = nc.gpsimd if MUL_ON_GPSIMD else nc.vector
            mul_eng.tensor_mul(
                o_tile[:, po:po + fd], anc_sb[ci][:, off:off + fd], inv[:, :fd])
            if gi % 2 == 1 or gs + fd == NC2:
                ms = (gs // OUT_TILE) * OUT_TILE
                mw = min(OUT_TILE, NC2 - ms)
                nc.sync.dma_start(out_d[:, ms:ms + mw], o_tile[:, :mw])

    nc.compile()
    return nc


_CACHED = None


def _get_program():
    global _CACHED
    if _CACHED is None:
        _CACHED = _build_program()
    return _CACHED


def kernel(coords, atom_nc, _trace=False, _trace_kwargs=None):
    coords = np.ascontiguousarray(np.asarray(coords, dtype=np.float32))
    atom_nc = np.ascontiguousarray(np.asarray(atom_nc, dtype=np.float32))
    assert coords.shape == (BATCH, N_ATOMS, 3)
    assert atom_nc.shape == (BATCH, NC2)

    nc = _get_program()
    smat = _build_smat16()
    c0, c1 = _split_coords2(coords)
    anc16 = (atom_nc * 2.0).astype(np.float16)  # fold the 2/K guard factor

    in_maps = []
    for core in range(N_CORES):
        b0 = core * BPC
        ct0 = np.ascontiguousarray(
            c0[b0:b0 + BPC].transpose(1, 2, 0).reshape(N_ATOMS, 3 * BPC))
        ct1 = np.ascontiguousarray(
            c1[b0:b0 + BPC].transpose(1, 2, 0).reshape(N_ATOMS, 3 * BPC))
        in_maps.append({
            "ct0": ct0,
            "ct1": ct1,
            "smat": smat,
            "atom_nc": anc16[b0:b0 + BPC],
        })

    kw = {}
    if _trace:
        kw["trace"] = True
        kw.update(_trace_kwargs or {})
    res = run_bass_kernel_spmd(nc, in_maps, core_ids=list(range(N_CORES)), **kw)
    out = np.concatenate(
        [np.asarray(r["out"]).astype(np.float32) for r in res.results], axis=0)
    if _trace:
        return out, res
    return out


if __name__ == "__main__":
    rng = np.random.default_rng(0)
    coords = (rng.standard_normal((BATCH, N_ATOMS, 3)) * 5.0).astype(np.float32)
    atom_nc = rng.uniform(1.0, 50.0, (BATCH, NC2)).astype(np.float32)
    out = kernel(coords, atom_nc)
    print(out.shape, out.dtype)
